# revision 7
# baseline (speedup 1.0000x reference)
"""DiffVG-style circle renderer on 8 Trainium2 NeuronCores.

Strategy: shard the 1024x1024 image by rows (128 rows per core). Each core
composites only the circles whose vertical span intersects its row band,
processing each circle front-to-back on a 224-column window around its
center:

    cov = sigmoid(arg)            arg ~= 2*(r - sqrt(dy^2+dx^2))  (see below)
    w   = T * cov                 (T = transmittance plane, init 1)
    C  += w * (alpha * color);  T -= alpha * w
Final:  rgb = C, a = 1 - T  (identical to the sequential 'over' scan).

The sigmoid argument is approximated per circle by an affine function of
t = d^2:  arg = a*t + b  with (a, b) least-max fitted on host against
2*(r - sqrt(t)).  This lets the PE matmul produce the *finished* sigmoid
argument (scale and bias folded into the operands), so ACT needs exactly
one table (sigmoid), no sqrt pass, and reads PSUM directly.

Engine split per circle:
  PE     arg = Ay[p] (+) Bx[x] outer-sum; two circles per matmul via a K=8
         block-diagonal layout (recentred hi/lo bf16 splits keep the
         argument accurate to ~6e-3)
  ACT    sigmoid straight from PSUM, one pair (448 cols) per op, bf16 out
  DVE    w = T*cov, T-MAC, R-MAC, G-MAC on dynamic 224-px windows (fp16
         2x mode; even window offsets keep 4B alignment), ordered
         [w_k, R_{k-1}, T_k, G_{k-1}] to pad same-engine RAW interlocks
  GPSIMD B-MAC (scalar_tensor_tensor)
Composite scalars come from a small per-core scal tensor (per-partition
scalar APs); the finish pass (RGBA interleave + alpha) runs on the host
from the 4 raw fp16 planes.
"""

import sys

if "/opt/trn_rl_repo" not in sys.path:
    sys.path.insert(0, "/opt/trn_rl_repo")

import numpy as np
import ml_dtypes

import concourse.bass as bass
import concourse.bacc as bacc
import concourse.mybir as mybir
from concourse.tile import TileContext, add_dep_helper
from concourse import bass_utils

H = 1024
W = 1024
ROWS = 128          # rows per core
N_CORES = 8
WIN = 224           # column window per circle (covers 2*(r+9) for r < 100)
MARGIN = 9.0
F32 = mybir.dt.float32
BF16 = mybir.dt.bfloat16
F16 = mybir.dt.float16
AF = mybir.ActivationFunctionType
OP = mybir.AluOpType
BF = ml_dtypes.bfloat16


def _build_core_inputs(centers, radii, colors, core):
    """Per-core circle list (slots ordered top-circle-first)."""
    y0 = ROWS * core
    cy = centers[:, 1].astype(np.float64)
    cx = centers[:, 0].astype(np.float64)
    r = radii.astype(np.float64)
    keep = (cy + r + MARGIN >= y0 + 0.5) & (cy - r - MARGIN <= y0 + ROWS - 0.5)
    idx = np.where(keep)[0][::-1]  # reversed: topmost (last-drawn) first
    return idx, cx[idx], cy[idx], r[idx], colors[idx].astype(np.float64)


def _hilo(x):
    hi = x.astype(BF)
    lo = (x - hi.astype(np.float64)).astype(BF)
    return hi, lo


def _fit_affine(r):
    """Least-max fit of sigmoid(a*t+b) to sigmoid(2*(r-sqrt(t))), t=d^2."""
    d = np.linspace(-11.0, 11.0, 353)
    t = (r + d) ** 2
    tgt = 1.0 / (1.0 + np.exp(2.0 * d))
    a0, b0 = -1.0 / r, r

    def err(u, v):
        arg = np.multiply.outer(u, a0 * t) + (b0 * u + v)[:, None]
        cov = 1.0 / (1.0 + np.exp(-arg))
        return np.abs(cov - tgt[None, :]).max(axis=1)

    u, v = 1.0, 0.0
    for span, n in ((0.06, 13), (0.012, 9), (0.002, 9)):
        us = u + np.linspace(-span, span, n)
        vs = v + np.linspace(-span * 8 * r / 20, span * 8 * r / 20, n)
        uu, vv = np.meshgrid(us, vs, indexing="ij")
        e = err(uu.ravel(), vv.ravel())
        k = int(np.argmin(e))
        u, v = uu.ravel()[k], vv.ravel()[k]
    return a0 * u, b0 * u + v


def make_inputs(centers, radii, colors, nc_slots):
    assert nc_slots % 8 == 0
    ins = []
    for core in range(N_CORES):
        y0 = ROWS * core
        idx, cx, cy, r, col = _build_core_inputs(centers, radii, colors, core)
        n = len(idx)
        assert n <= nc_slots
        offs = np.zeros((1, nc_slots), np.int32)
        # per-circle scalars, replicated down partitions:
        # [4k+0]=-alpha, [4k+1]=alpha*cr, [4k+2]=alpha*cg, [4k+3]=alpha*cb
        scal = np.zeros((ROWS, nc_slots * 4), np.float32)
        # two circles (a, b) share one K=8 matmul: lhsT rows 0-3 belong to
        # a, rows 4-7 to b; rhs zero-masks the other circle's columns.
        lhsT = np.zeros((8, (nc_slots // 2) * ROWS), BF)
        rhs = np.zeros((8, nc_slots * WIN), BF)

        p = np.arange(ROWS, dtype=np.float64)
        j = np.arange(WIN, dtype=np.float64)
        for k in range(n):
            off = int(np.clip(np.floor(cx[k]) - 112.0, 0.0, float(W - WIN)))
            off &= ~1  # even: keeps bf16 windows 4B-aligned for 2x DVE mode
            offs[0, k] = off
            a_k, b_k = _fit_affine(r[k])
            ya = a_k * (y0 + p + 0.5 - cy[k]) ** 2 + b_k
            xa = a_k * (off + j + 0.5 - cx[k]) ** 2
            # recentre so each hi/lo bf16 split carries a small range
            c = -0.5 * (ya.max() + ya.min())
            ya = ya + c
            xa = xa - c
            alpha = col[k, 3]
            scal[:, 4 * k + 0] = -alpha
            scal[:, 4 * k + 1] = alpha * col[k, 0]
            scal[:, 4 * k + 2] = alpha * col[k, 1]
            scal[:, 4 * k + 3] = alpha * col[k, 2]
            yh, yl = _hilo(ya)
            xh, xl = _hilo(xa)
            pair, half = divmod(k, 2)
            rbase = 4 * half
            ls = slice(pair * ROWS, (pair + 1) * ROWS)
            lhsT[rbase + 0, ls] = yh
            lhsT[rbase + 1, ls] = yl
            lhsT[rbase + 2, ls] = 1.0
            lhsT[rbase + 3, ls] = 1.0
            rs = slice(k * WIN, (k + 1) * WIN)
            rhs[rbase + 0, rs] = 1.0
            rhs[rbase + 1, rs] = 1.0
            rhs[rbase + 2, rs] = xh
            rhs[rbase + 3, rs] = xl
        ins.append({"offs": offs, "scal": scal, "lhsT": lhsT, "rhs": rhs})
    return ins


def build_nc(nc_slots):
    assert nc_slots % 8 == 0
    nc = bacc.Bacc("TRN2", target_bir_lowering=False, debug=False,
                   num_devices=N_CORES)
    offs_d = nc.dram_tensor("offs", [1, nc_slots], mybir.dt.int32,
                            kind="ExternalInput").ap()
    scal_d = nc.dram_tensor("scal", [ROWS, nc_slots * 4], F32,
                            kind="ExternalInput").ap()
    lhsT_d = nc.dram_tensor("lhsT", [8, (nc_slots // 2) * ROWS], BF16,
                            kind="ExternalInput").ap()
    rhs_d = nc.dram_tensor("rhs", [8, nc_slots * WIN], BF16,
                           kind="ExternalInput").ap()
    out_d = nc.dram_tensor("out", [ROWS, W * 4], F16,
                           kind="ExternalOutput").ap()

    with TileContext(nc) as tc:
        # persistent state (bf16 planes)
        T = nc.alloc_sbuf_tensor("T", [ROWS, W], F16).ap()
        CR = nc.alloc_sbuf_tensor("CR", [ROWS, W], F16).ap()
        CG = nc.alloc_sbuf_tensor("CG", [ROWS, W], F16).ap()
        CB = nc.alloc_sbuf_tensor("CB", [ROWS, W], F16).ap()
        offs_sb = nc.alloc_sbuf_tensor("offs_sb", [1, nc_slots],
                                       mybir.dt.int32).ap()
        scal_sb = nc.alloc_sbuf_tensor("scal_sb", [ROWS, nc_slots * 4],
                                       F32).ap()

        nc.sync.dma_start(offs_sb, offs_d)
        nc.sync.dma_start(scal_sb, scal_d)
        nc.vector.memset(T, 1.0)
        nc.vector.memset(CR, 0.0)
        nc.gpsimd.memset(CG, 0.0)
        nc.gpsimd.memset(CB, 0.0)

        with (
            tc.tile_pool(name="psum", bufs=2, space="PSUM") as psum_pool,
            tc.tile_pool(name="ops", bufs=3) as oppool,
            tc.tile_pool(name="cov", bufs=6) as covpool,
            tc.tile_pool(name="w", bufs=6) as wpool,
            tc.tile_pool(name="tmpb", bufs=6) as bpool,
            tc.tile_pool(name="mcov", bufs=12) as mpool,
        ):
            prev_v = None
            prev_g = None
            # pending R/G MAC for the previous circle (emitted one circle
            # late so the serial T chain never reads a value written by
            # the immediately preceding DVE instruction)
            pend = None

            for g8 in range(0, nc_slots, 8):
                lh_t = oppool.tile([8, 4 * ROWS], BF16, tag="lh")
                rh_t = oppool.tile([8, 8 * WIN], BF16, tag="rh")
                p0 = g8 // 2
                nc.sync.dma_start(lh_t, lhsT_d[:, p0 * ROWS:(p0 + 4) * ROWS])
                nc.sync.dma_start(rh_t, rhs_d[:, g8 * WIN:(g8 + 8) * WIN])

                pt = psum_pool.tile([ROWS, 4 * 512], F32)
                for i in range(4):
                    nc.tensor.matmul(
                        pt[:, i * 512:i * 512 + 2 * WIN],
                        lh_t[:, i * ROWS:(i + 1) * ROWS],
                        rh_t[:, i * 2 * WIN:(i + 1) * 2 * WIN],
                        start=True, stop=True)
                cov8 = covpool.tile([ROWS, 8 * WIN], F16)
                pview = pt.rearrange("p (b f) -> p b f", f=512)
                cview = cov8.rearrange("p (b f) -> p b f", f=2 * WIN)
                nc.scalar.activation(cview, pview[:, :, :2 * WIN], AF.Sigmoid)
                # mcov = 1 - alpha*cov on GPSIMD (independent of the T chain)
                mcovs = []
                for i in range(8):
                    k = g8 + i
                    mc = mpool.tile([ROWS, WIN], F16)
                    nc.gpsimd.tensor_scalar(
                        mc, cov8[:, i * WIN:(i + 1) * WIN],
                        scal_sb[:, 4 * k + 0:4 * k + 1], 1.0,
                        OP.mult, OP.add)
                    mcovs.append(mc)

                vregs = [nc.vector.alloc_register(f"offv_{g8}_{i}")
                         for i in range(8)]
                liv = nc.vector.reg_load(vregs, offs_sb[0:1, g8:g8 + 8])
                if prev_v is not None:
                    add_dep_helper(liv.ins, prev_v.ins, sync=False,
                                   reason="reg pressure")
                voff = [nc.vector.snap(rg, donate=True,
                                       min_val=0, max_val=W - WIN)
                        for rg in vregs]
                gregs = [nc.gpsimd.alloc_register(f"offg_{g8}_{i}")
                         for i in range(8)]
                lig = nc.gpsimd.reg_load(gregs, offs_sb[0:1, g8:g8 + 8])
                if prev_g is not None:
                    add_dep_helper(lig.ins, prev_g.ins, sync=False,
                                   reason="reg pressure")
                goff = [nc.gpsimd.snap(rg, donate=True,
                                       min_val=0, max_val=W - WIN)
                        for rg in gregs]

                for i in range(8):
                    k = g8 + i
                    cov = cov8[:, i * WIN:(i + 1) * WIN]
                    # DVE order: w_k, R_{k-1}, T_k, G_{k-1}
                    tw = T[:, bass.ds(voff[i], WIN)]
                    w = wpool.tile([ROWS, WIN], F16)
                    nc.vector.tensor_tensor(w, tw, cov, OP.mult)
                    tmpb = bpool.tile([ROWS, WIN], F16)
                    nc.scalar.activation(
                        tmpb, w, AF.Copy,
                        scale=scal_sb[:, 4 * k + 3:4 * k + 4])
                    cbw = CB[:, bass.ds(goff[i], WIN)]
                    prev_g = nc.gpsimd.tensor_tensor(cbw, cbw, tmpb, OP.add)
                    wp, kp, offp = (pend if pend is not None
                                    else (None, None, None))
                    if wp is not None:
                        crw = CR[:, bass.ds(offp, WIN)]
                        nc.vector.scalar_tensor_tensor(
                            crw, wp, scal_sb[:, 4 * kp + 1:4 * kp + 2], crw,
                            OP.mult, OP.add)
                    nc.vector.tensor_tensor(tw, tw, mcovs[i], OP.mult)
                    if wp is not None:
                        cgw = CG[:, bass.ds(offp, WIN)]
                        prev_v = nc.vector.scalar_tensor_tensor(
                            cgw, wp, scal_sb[:, 4 * kp + 2:4 * kp + 3], cgw,
                            OP.mult, OP.add)
                    pend = (w, k, voff[i])

            # flush the last circle's R/G MACs
            if pend is not None:
                wp, kp, offp = pend
                crw = CR[:, bass.ds(offp, WIN)]
                nc.vector.scalar_tensor_tensor(
                    crw, wp, scal_sb[:, 4 * kp + 1:4 * kp + 2], crw,
                    OP.mult, OP.add)
                cgw = CG[:, bass.ds(offp, WIN)]
                nc.vector.scalar_tensor_tensor(
                    cgw, wp, scal_sb[:, 4 * kp + 2:4 * kp + 3], cgw,
                    OP.mult, OP.add)
                pend = None

        # ---------- store raw planes; host interleaves + computes alpha ---
        nc.sync.dma_start(out_d[:, 0 * W:1 * W], CR)
        nc.sync.dma_start(out_d[:, 1 * W:2 * W], CG)
        nc.sync.dma_start(out_d[:, 2 * W:3 * W], CB)
        nc.sync.dma_start(out_d[:, 3 * W:4 * W], T)

    nc.compile()
    return nc


_CACHE = {}


def _get_nc(nc_slots):
    if nc_slots not in _CACHE:
        _CACHE[nc_slots] = build_nc(nc_slots)
    return _CACHE[nc_slots]


def kernel(centers, radii, colors):
    centers = np.asarray(centers, np.float32)
    radii = np.asarray(radii, np.float32)
    colors = np.asarray(colors, np.float32)

    counts = []
    for core in range(N_CORES):
        idx, *_ = _build_core_inputs(centers, radii, colors, core)
        counts.append(len(idx))
    nc_slots = max(8, ((max(counts) + 7) // 8) * 8)

    nc = _get_nc(nc_slots)
    ins = make_inputs(centers, radii, colors, nc_slots)
    res = bass_utils.run_bass_kernel_spmd(nc, ins, list(range(N_CORES)),
                                          trace=False)
    out = np.zeros((H, W, 4), np.float32)
    for c in range(N_CORES):
        planes = np.asarray(res.results[c]["out"]).astype(np.float32)
        planes = planes.reshape(ROWS, 4, W)
        sl = slice(c * ROWS, (c + 1) * ROWS)
        out[sl, :, 0] = planes[:, 0]
        out[sl, :, 1] = planes[:, 1]
        out[sl, :, 2] = planes[:, 2]
        out[sl, :, 3] = 1.0 - planes[:, 3]
    return out


# revision 9
# speedup vs baseline: 1.0091x; 1.0091x over previous
"""DiffVG-style circle renderer on 8 Trainium2 NeuronCores.

Strategy: shard the 1024x1024 image by rows (128 rows per core). Each core
composites only the circles whose vertical span intersects its row band,
processing each circle front-to-back on a 224-column window around its
center:

    cov = sigmoid(arg)            arg ~= 2*(r - sqrt(dy^2+dx^2))  (see below)
    w   = T * cov                 (T = transmittance plane, init 1)
    C  += w * (alpha * color);  T -= alpha * w
Final:  rgb = C, a = 1 - T  (identical to the sequential 'over' scan).

The sigmoid argument is approximated per circle by an affine function of
t = d^2:  arg = a*t + b  with (a, b) least-max fitted on host against
2*(r - sqrt(t)).  This lets the PE matmul produce the *finished* sigmoid
argument (scale and bias folded into the operands), so ACT needs exactly
one table (sigmoid), no sqrt pass, and reads PSUM directly.

Engine split per circle:
  PE     arg = Ay[p] (+) Bx[x] outer-sum; two circles per matmul via a K=8
         block-diagonal layout (recentred hi/lo bf16 splits keep the
         argument accurate to ~6e-3)
  ACT    sigmoid straight from PSUM, one pair (448 cols) per op, bf16 out
  DVE    w = T*cov, T-MAC, R-MAC, G-MAC on dynamic 224-px windows (fp16
         2x mode; even window offsets keep 4B alignment), ordered
         [w_k, R_{k-1}, T_k, G_{k-1}] to pad same-engine RAW interlocks
  GPSIMD B-MAC (scalar_tensor_tensor)
Composite scalars come from a small per-core scal tensor (per-partition
scalar APs); the finish pass (RGBA interleave + alpha) runs on the host
from the 4 raw fp16 planes.
"""

import sys

if "/opt/trn_rl_repo" not in sys.path:
    sys.path.insert(0, "/opt/trn_rl_repo")

import numpy as np
import ml_dtypes

import concourse.bass as bass
import concourse.bacc as bacc
import concourse.mybir as mybir
from concourse.tile import TileContext, add_dep_helper
from concourse import bass_utils

H = 1024
W = 1024
ROWS = 128          # rows per core
N_CORES = 8
WIN = 224           # column window per circle (covers 2*(r+9) for r < 100)
MARGIN = 9.0
F32 = mybir.dt.float32
BF16 = mybir.dt.bfloat16
F16 = mybir.dt.float16
AF = mybir.ActivationFunctionType
OP = mybir.AluOpType
BF = ml_dtypes.bfloat16


def _build_core_inputs(centers, radii, colors, core):
    """Per-core circle list (slots ordered top-circle-first)."""
    y0 = ROWS * core
    cy = centers[:, 1].astype(np.float64)
    cx = centers[:, 0].astype(np.float64)
    r = radii.astype(np.float64)
    keep = (cy + r + MARGIN >= y0 + 0.5) & (cy - r - MARGIN <= y0 + ROWS - 0.5)
    idx = np.where(keep)[0][::-1]  # reversed: topmost (last-drawn) first
    return idx, cx[idx], cy[idx], r[idx], colors[idx].astype(np.float64)


def _hilo(x):
    hi = x.astype(BF)
    lo = (x - hi.astype(np.float64)).astype(BF)
    return hi, lo


def _fit_affine(r):
    """Least-max fit of sigmoid(a*t+b) to sigmoid(2*(r-sqrt(t))), t=d^2."""
    d = np.linspace(-11.0, 11.0, 353)
    t = (r + d) ** 2
    tgt = 1.0 / (1.0 + np.exp(2.0 * d))
    a0, b0 = -1.0 / r, r

    def err(u, v):
        arg = np.multiply.outer(u, a0 * t) + (b0 * u + v)[:, None]
        cov = 1.0 / (1.0 + np.exp(-arg))
        return np.abs(cov - tgt[None, :]).max(axis=1)

    u, v = 1.0, 0.0
    for span, n in ((0.06, 13), (0.012, 9), (0.002, 9)):
        us = u + np.linspace(-span, span, n)
        vs = v + np.linspace(-span * 8 * r / 20, span * 8 * r / 20, n)
        uu, vv = np.meshgrid(us, vs, indexing="ij")
        e = err(uu.ravel(), vv.ravel())
        k = int(np.argmin(e))
        u, v = uu.ravel()[k], vv.ravel()[k]
    return a0 * u, b0 * u + v


def make_inputs(centers, radii, colors, nc_slots):
    assert nc_slots % 8 == 0
    ins = []
    for core in range(N_CORES):
        y0 = ROWS * core
        idx, cx, cy, r, col = _build_core_inputs(centers, radii, colors, core)
        n = len(idx)
        assert n <= nc_slots
        offs = np.zeros((1, nc_slots), np.int32)
        # per-circle scalars, replicated down partitions:
        # [4k+0]=-alpha, [4k+1]=alpha*cr, [4k+2]=alpha*cg, [4k+3]=alpha*cb
        scal = np.zeros((ROWS, nc_slots * 4), np.float32)
        # two circles (a, b) share one K=8 matmul: lhsT rows 0-3 belong to
        # a, rows 4-7 to b; rhs zero-masks the other circle's columns.
        lhsT = np.zeros((8, (nc_slots // 2) * ROWS), BF)
        rhs = np.zeros((8, nc_slots * WIN), BF)

        p = np.arange(ROWS, dtype=np.float64)
        j = np.arange(WIN, dtype=np.float64)
        for k in range(n):
            off = int(np.clip(np.floor(cx[k]) - 112.0, 0.0, float(W - WIN)))
            off &= ~1  # even: keeps bf16 windows 4B-aligned for 2x DVE mode
            offs[0, k] = off
            a_k, b_k = _fit_affine(r[k])
            ya = a_k * (y0 + p + 0.5 - cy[k]) ** 2 + b_k
            xa = a_k * (off + j + 0.5 - cx[k]) ** 2
            # recentre so each hi/lo bf16 split carries a small range
            c = -0.5 * (ya.max() + ya.min())
            ya = ya + c
            xa = xa - c
            alpha = col[k, 3]
            scal[:, 4 * k + 0] = -alpha
            scal[:, 4 * k + 1] = alpha * col[k, 0]
            scal[:, 4 * k + 2] = alpha * col[k, 1]
            scal[:, 4 * k + 3] = alpha * col[k, 2]
            yh, yl = _hilo(ya)
            xh, xl = _hilo(xa)
            pair, half = divmod(k, 2)
            rbase = 4 * half
            ls = slice(pair * ROWS, (pair + 1) * ROWS)
            lhsT[rbase + 0, ls] = yh
            lhsT[rbase + 1, ls] = yl
            lhsT[rbase + 2, ls] = 1.0
            lhsT[rbase + 3, ls] = 1.0
            rs = slice(k * WIN, (k + 1) * WIN)
            rhs[rbase + 0, rs] = 1.0
            rhs[rbase + 1, rs] = 1.0
            rhs[rbase + 2, rs] = xh
            rhs[rbase + 3, rs] = xl
        ins.append({"offs": offs, "scal": scal, "lhsT": lhsT, "rhs": rhs})
    return ins


def build_nc(nc_slots):
    assert nc_slots % 8 == 0
    nc = bacc.Bacc("TRN2", target_bir_lowering=False, debug=False,
                   num_devices=N_CORES)
    offs_d = nc.dram_tensor("offs", [1, nc_slots], mybir.dt.int32,
                            kind="ExternalInput").ap()
    scal_d = nc.dram_tensor("scal", [ROWS, nc_slots * 4], F32,
                            kind="ExternalInput").ap()
    lhsT_d = nc.dram_tensor("lhsT", [8, (nc_slots // 2) * ROWS], BF16,
                            kind="ExternalInput").ap()
    rhs_d = nc.dram_tensor("rhs", [8, nc_slots * WIN], BF16,
                           kind="ExternalInput").ap()
    out_d = nc.dram_tensor("out", [ROWS, W * 4], F16,
                           kind="ExternalOutput").ap()

    with TileContext(nc) as tc:
        # persistent state (bf16 planes)
        T = nc.alloc_sbuf_tensor("T", [ROWS, W], F16).ap()
        CR = nc.alloc_sbuf_tensor("CR", [ROWS, W], F16).ap()
        CG = nc.alloc_sbuf_tensor("CG", [ROWS, W], F16).ap()
        CB = nc.alloc_sbuf_tensor("CB", [ROWS, W], F16).ap()
        offs_sb = nc.alloc_sbuf_tensor("offs_sb", [1, nc_slots],
                                       mybir.dt.int32).ap()
        scal_sb = nc.alloc_sbuf_tensor("scal_sb", [ROWS, nc_slots * 4],
                                       F32).ap()

        nc.sync.dma_start(offs_sb, offs_d)
        nc.sync.dma_start(scal_sb, scal_d)
        nc.vector.memset(T, 1.0)
        nc.vector.memset(CR, 0.0)
        nc.gpsimd.memset(CG, 0.0)
        nc.gpsimd.memset(CB, 0.0)

        with (
            tc.tile_pool(name="psum", bufs=2, space="PSUM") as psum_pool,
            tc.tile_pool(name="ops", bufs=3) as oppool,
            tc.tile_pool(name="cov", bufs=6) as covpool,
            tc.tile_pool(name="w", bufs=6) as wpool,
            tc.tile_pool(name="tmpb", bufs=6) as bpool,
        ):
            prev_v = None
            prev_g = None
            # pending R/G MAC for the previous circle (emitted one circle
            # late so the serial T chain never reads a value written by
            # the immediately preceding DVE instruction)
            pend = None

            def front(g8):
                """DMA + matmul + batched sigmoid for one group of 8."""
                lh_t = oppool.tile([8, 4 * ROWS], BF16, tag="lh")
                rh_t = oppool.tile([8, 8 * WIN], BF16, tag="rh")
                p0 = g8 // 2
                nc.sync.dma_start(lh_t, lhsT_d[:, p0 * ROWS:(p0 + 4) * ROWS])
                nc.sync.dma_start(rh_t, rhs_d[:, g8 * WIN:(g8 + 8) * WIN])
                pt = psum_pool.tile([ROWS, 4 * 512], F32)
                for i in range(4):
                    nc.tensor.matmul(
                        pt[:, i * 512:i * 512 + 2 * WIN],
                        lh_t[:, i * ROWS:(i + 1) * ROWS],
                        rh_t[:, i * 2 * WIN:(i + 1) * 2 * WIN],
                        start=True, stop=True)
                cov8 = covpool.tile([ROWS, 8 * WIN], F16)
                pview = pt.rearrange("p (b f) -> p b f", f=512)
                cview = cov8.rearrange("p (b f) -> p b f", f=2 * WIN)
                nc.scalar.activation(cview, pview[:, :, :2 * WIN], AF.Sigmoid)
                return cov8

            cov_next = front(0)
            voff = [None] * nc_slots
            goff = [None] * nc_slots

            for g8 in range(0, nc_slots, 8):
                cov8 = cov_next
                if g8 + 8 < nc_slots:
                    cov_next = front(g8 + 8)

                if g8 % 16 == 0:
                    nk = min(16, nc_slots - g8)
                    vregs = [nc.vector.alloc_register(f"offv_{g8}_{i}")
                             for i in range(nk)]
                    liv = nc.vector.reg_load(vregs,
                                             offs_sb[0:1, g8:g8 + nk])
                    if prev_v is not None:
                        add_dep_helper(liv.ins, prev_v.ins, sync=False,
                                       reason="reg pressure")
                    for i in range(nk):
                        voff[g8 + i] = nc.vector.snap(
                            vregs[i], donate=True,
                            min_val=0, max_val=W - WIN)
                    gregs = [nc.gpsimd.alloc_register(f"offg_{g8}_{i}")
                             for i in range(nk)]
                    lig = nc.gpsimd.reg_load(gregs,
                                             offs_sb[0:1, g8:g8 + nk])
                    if prev_g is not None:
                        add_dep_helper(lig.ins, prev_g.ins, sync=False,
                                       reason="reg pressure")
                    for i in range(nk):
                        goff[g8 + i] = nc.gpsimd.snap(
                            gregs[i], donate=True,
                            min_val=0, max_val=W - WIN)

                for i in range(8):
                    k = g8 + i
                    cov = cov8[:, i * WIN:(i + 1) * WIN]
                    # DVE order: w_k, R_{k-1}, T_k, G_{k-1}
                    tw = T[:, bass.ds(voff[k], WIN)]
                    w = wpool.tile([ROWS, WIN], F16)
                    nc.vector.tensor_tensor(w, tw, cov, OP.mult)
                    tmpb = bpool.tile([ROWS, WIN], F16)
                    nc.scalar.activation(
                        tmpb, w, AF.Copy,
                        scale=scal_sb[:, 4 * k + 3:4 * k + 4])
                    cbw = CB[:, bass.ds(goff[k], WIN)]
                    prev_g = nc.gpsimd.tensor_tensor(cbw, cbw, tmpb, OP.add)
                    wp, kp, offp = (pend if pend is not None
                                    else (None, None, None))
                    if wp is not None:
                        crw = CR[:, bass.ds(offp, WIN)]
                        nc.vector.scalar_tensor_tensor(
                            crw, wp, scal_sb[:, 4 * kp + 1:4 * kp + 2], crw,
                            OP.mult, OP.add)
                    nc.vector.scalar_tensor_tensor(
                        tw, w, scal_sb[:, 4 * k + 0:4 * k + 1], tw,
                        OP.mult, OP.add)
                    if wp is not None:
                        cgw = CG[:, bass.ds(offp, WIN)]
                        prev_v = nc.vector.scalar_tensor_tensor(
                            cgw, wp, scal_sb[:, 4 * kp + 2:4 * kp + 3], cgw,
                            OP.mult, OP.add)
                    pend = (w, k, voff[k])

            # flush the last circle's R/G MACs
            if pend is not None:
                wp, kp, offp = pend
                crw = CR[:, bass.ds(offp, WIN)]
                nc.vector.scalar_tensor_tensor(
                    crw, wp, scal_sb[:, 4 * kp + 1:4 * kp + 2], crw,
                    OP.mult, OP.add)
                cgw = CG[:, bass.ds(offp, WIN)]
                nc.vector.scalar_tensor_tensor(
                    cgw, wp, scal_sb[:, 4 * kp + 2:4 * kp + 3], cgw,
                    OP.mult, OP.add)
                pend = None

        # ---------- store raw planes; host interleaves + computes alpha ---
        nc.sync.dma_start(out_d[:, 0 * W:1 * W], CR)
        nc.sync.dma_start(out_d[:, 1 * W:2 * W], CG)
        nc.sync.dma_start(out_d[:, 2 * W:3 * W], CB)
        nc.sync.dma_start(out_d[:, 3 * W:4 * W], T)

    nc.compile()
    return nc


_CACHE = {}


def _get_nc(nc_slots):
    if nc_slots not in _CACHE:
        _CACHE[nc_slots] = build_nc(nc_slots)
    return _CACHE[nc_slots]


def kernel(centers, radii, colors):
    centers = np.asarray(centers, np.float32)
    radii = np.asarray(radii, np.float32)
    colors = np.asarray(colors, np.float32)

    counts = []
    for core in range(N_CORES):
        idx, *_ = _build_core_inputs(centers, radii, colors, core)
        counts.append(len(idx))
    nc_slots = max(8, ((max(counts) + 7) // 8) * 8)

    nc = _get_nc(nc_slots)
    ins = make_inputs(centers, radii, colors, nc_slots)
    res = bass_utils.run_bass_kernel_spmd(nc, ins, list(range(N_CORES)),
                                          trace=False)
    out = np.zeros((H, W, 4), np.float32)
    for c in range(N_CORES):
        planes = np.asarray(res.results[c]["out"]).astype(np.float32)
        planes = planes.reshape(ROWS, 4, W)
        sl = slice(c * ROWS, (c + 1) * ROWS)
        out[sl, :, 0] = planes[:, 0]
        out[sl, :, 1] = planes[:, 1]
        out[sl, :, 2] = planes[:, 2]
        out[sl, :, 3] = 1.0 - planes[:, 3]
    return out


# revision 10
# speedup vs baseline: 1.0205x; 1.0113x over previous
"""DiffVG-style circle renderer on 8 Trainium2 NeuronCores.

Strategy: shard the 1024x1024 image by rows (128 rows per core). Each core
composites only the circles whose vertical span intersects its row band,
processing each circle front-to-back on a 224-column window around its
center:

    cov = sigmoid(arg)            arg ~= 2*(r - sqrt(dy^2+dx^2))  (see below)
    w   = T * cov                 (T = transmittance plane, init 1)
    C  += w * (alpha * color);  T -= alpha * w
Final:  rgb = C, a = 1 - T  (identical to the sequential 'over' scan).

The sigmoid argument is approximated per circle by an affine function of
t = d^2:  arg = a*t + b  with (a, b) least-max fitted on host against
2*(r - sqrt(t)).  This lets the PE matmul produce the *finished* sigmoid
argument (scale and bias folded into the operands), so ACT needs exactly
one table (sigmoid), no sqrt pass, and reads PSUM directly.

Engine split per circle:
  PE     arg = Ay[p] (+) Bx[x] outer-sum; two circles per matmul via a K=8
         block-diagonal layout (recentred hi/lo bf16 splits keep the
         argument accurate to ~6e-3)
  ACT    sigmoid straight from PSUM, one pair (448 cols) per op, bf16 out
  DVE    w = T*cov, T-MAC, R-MAC, G-MAC on dynamic 224-px windows (fp16
         2x mode; even window offsets keep 4B alignment), ordered
         [w_k, R_{k-1}, T_k, G_{k-1}] to pad same-engine RAW interlocks
  GPSIMD B-MAC (scalar_tensor_tensor)
Composite scalars come from a small per-core scal tensor (per-partition
scalar APs); the finish pass (RGBA interleave + alpha) runs on the host
from the 4 raw fp16 planes.
"""

import sys

if "/opt/trn_rl_repo" not in sys.path:
    sys.path.insert(0, "/opt/trn_rl_repo")

import numpy as np
import ml_dtypes

import concourse.bass as bass
import concourse.bacc as bacc
import concourse.mybir as mybir
from concourse.tile import TileContext, add_dep_helper
from concourse import bass_utils

H = 1024
W = 1024
ROWS = 128          # rows per core
N_CORES = 8
WIN = 224           # column window per circle (covers 2*(r+9) for r < 100)
MARGIN = 9.0
F32 = mybir.dt.float32
BF16 = mybir.dt.bfloat16
F16 = mybir.dt.float16
AF = mybir.ActivationFunctionType
OP = mybir.AluOpType
BF = ml_dtypes.bfloat16


def _build_core_inputs(centers, radii, colors, core):
    """Per-core circle list (slots ordered top-circle-first)."""
    y0 = ROWS * core
    cy = centers[:, 1].astype(np.float64)
    cx = centers[:, 0].astype(np.float64)
    r = radii.astype(np.float64)
    keep = (cy + r + MARGIN >= y0 + 0.5) & (cy - r - MARGIN <= y0 + ROWS - 0.5)
    idx = np.where(keep)[0][::-1]  # reversed: topmost (last-drawn) first
    return idx, cx[idx], cy[idx], r[idx], colors[idx].astype(np.float64)


def _hilo(x):
    hi = x.astype(BF)
    lo = (x - hi.astype(np.float64)).astype(BF)
    return hi, lo


def _fit_affine(r):
    """Least-max fit of sigmoid(a*t+b) to sigmoid(2*(r-sqrt(t))), t=d^2."""
    d = np.linspace(-11.0, 11.0, 353)
    t = (r + d) ** 2
    tgt = 1.0 / (1.0 + np.exp(2.0 * d))
    a0, b0 = -1.0 / r, r

    def err(u, v):
        arg = np.multiply.outer(u, a0 * t) + (b0 * u + v)[:, None]
        cov = 1.0 / (1.0 + np.exp(-arg))
        return np.abs(cov - tgt[None, :]).max(axis=1)

    u, v = 1.0, 0.0
    for span, n in ((0.06, 13), (0.012, 9), (0.002, 9)):
        us = u + np.linspace(-span, span, n)
        vs = v + np.linspace(-span * 8 * r / 20, span * 8 * r / 20, n)
        uu, vv = np.meshgrid(us, vs, indexing="ij")
        e = err(uu.ravel(), vv.ravel())
        k = int(np.argmin(e))
        u, v = uu.ravel()[k], vv.ravel()[k]
    return a0 * u, b0 * u + v


def make_inputs(centers, radii, colors, nc_slots):
    assert nc_slots % 8 == 0
    ins = []
    for core in range(N_CORES):
        y0 = ROWS * core
        idx, cx, cy, r, col = _build_core_inputs(centers, radii, colors, core)
        n = len(idx)
        assert n <= nc_slots
        offs = np.zeros((1, nc_slots), np.int32)
        # per-circle scalars, replicated down partitions:
        # [4k+0]=-alpha, [4k+1]=alpha*cr, [4k+2]=alpha*cg, [4k+3]=alpha*cb
        scal = np.zeros((ROWS, nc_slots * 4), np.float32)
        # two circles (a, b) share one K=8 matmul: lhsT rows 0-3 belong to
        # a, rows 4-7 to b; rhs zero-masks the other circle's columns.
        lhsT = np.zeros((8, (nc_slots // 2) * ROWS), BF)
        rhs = np.zeros((8, nc_slots * WIN), BF)

        p = np.arange(ROWS, dtype=np.float64)
        j = np.arange(WIN, dtype=np.float64)
        for k in range(n):
            off = int(np.clip(np.floor(cx[k]) - 112.0, 0.0, float(W - WIN)))
            off &= ~1  # even: keeps bf16 windows 4B-aligned for 2x DVE mode
            offs[0, k] = off
            a_k, b_k = _fit_affine(r[k])
            ya = a_k * (y0 + p + 0.5 - cy[k]) ** 2 + b_k
            xa = a_k * (off + j + 0.5 - cx[k]) ** 2
            # recentre so each hi/lo bf16 split carries a small range
            c = -0.5 * (ya.max() + ya.min())
            ya = ya + c
            xa = xa - c
            alpha = col[k, 3]
            scal[:, 4 * k + 0] = -alpha
            scal[:, 4 * k + 1] = alpha * col[k, 0]
            scal[:, 4 * k + 2] = alpha * col[k, 1]
            scal[:, 4 * k + 3] = alpha * col[k, 2]
            yh, yl = _hilo(ya)
            xh, xl = _hilo(xa)
            pair, half = divmod(k, 2)
            rbase = 4 * half
            ls = slice(pair * ROWS, (pair + 1) * ROWS)
            lhsT[rbase + 0, ls] = yh
            lhsT[rbase + 1, ls] = yl
            lhsT[rbase + 2, ls] = 1.0
            lhsT[rbase + 3, ls] = 1.0
            rs = slice(k * WIN, (k + 1) * WIN)
            rhs[rbase + 0, rs] = 1.0
            rhs[rbase + 1, rs] = 1.0
            rhs[rbase + 2, rs] = xh
            rhs[rbase + 3, rs] = xl
        ins.append({"offs": offs, "scal": scal, "lhsT": lhsT, "rhs": rhs})
    return ins


def build_nc(nc_slots):
    assert nc_slots % 8 == 0
    nc = bacc.Bacc("TRN2", target_bir_lowering=False, debug=False,
                   num_devices=N_CORES)
    offs_d = nc.dram_tensor("offs", [1, nc_slots], mybir.dt.int32,
                            kind="ExternalInput").ap()
    scal_d = nc.dram_tensor("scal", [ROWS, nc_slots * 4], F32,
                            kind="ExternalInput").ap()
    lhsT_d = nc.dram_tensor("lhsT", [8, (nc_slots // 2) * ROWS], BF16,
                            kind="ExternalInput").ap()
    rhs_d = nc.dram_tensor("rhs", [8, nc_slots * WIN], BF16,
                           kind="ExternalInput").ap()
    out_d = nc.dram_tensor("out", [ROWS, W * 4], F16,
                           kind="ExternalOutput").ap()

    with TileContext(nc) as tc:
        # persistent state (bf16 planes)
        T = nc.alloc_sbuf_tensor("T", [ROWS, W], F16).ap()
        CR = nc.alloc_sbuf_tensor("CR", [ROWS, W], F16).ap()
        CG = nc.alloc_sbuf_tensor("CG", [ROWS, W], F16).ap()
        CB = nc.alloc_sbuf_tensor("CB", [ROWS, W], F16).ap()
        offs_sb = nc.alloc_sbuf_tensor("offs_sb", [1, nc_slots],
                                       mybir.dt.int32).ap()
        scal_sb = nc.alloc_sbuf_tensor("scal_sb", [ROWS, nc_slots * 4],
                                       F32).ap()

        nc.sync.dma_start(offs_sb, offs_d)
        nc.sync.dma_start(scal_sb, scal_d)
        nc.vector.memset(T, 1.0)
        nc.vector.memset(CR, 0.0)
        nc.gpsimd.memset(CG, 0.0)
        nc.gpsimd.memset(CB, 0.0)

        with (
            tc.tile_pool(name="psum", bufs=6, space="PSUM") as psum_pool,
            tc.tile_pool(name="ops", bufs=3) as oppool,
            tc.tile_pool(name="cov", bufs=6) as covpool,
            tc.tile_pool(name="w", bufs=6) as wpool,
            tc.tile_pool(name="tmpb", bufs=6) as bpool,
        ):
            prev_v = None
            prev_g = None
            # pending R/G MAC for the previous circle (emitted one circle
            # late so the serial T chain never reads a value written by
            # the immediately preceding DVE instruction)
            pend = None

            for g8 in range(0, nc_slots, 8):
                lh_t = oppool.tile([8, 4 * ROWS], BF16, tag="lh")
                rh_t = oppool.tile([8, 8 * WIN], BF16, tag="rh")
                p0 = g8 // 2
                nc.sync.dma_start(lh_t, lhsT_d[:, p0 * ROWS:(p0 + 4) * ROWS])
                nc.sync.dma_start(rh_t, rhs_d[:, g8 * WIN:(g8 + 8) * WIN])

                covs = []
                for i in range(4):
                    pt = psum_pool.tile([ROWS, 2 * WIN], F32)
                    nc.tensor.matmul(
                        pt,
                        lh_t[:, i * ROWS:(i + 1) * ROWS],
                        rh_t[:, i * 2 * WIN:(i + 1) * 2 * WIN],
                        start=True, stop=True)
                    cv = covpool.tile([ROWS, 2 * WIN], F16)
                    nc.scalar.activation(cv, pt, AF.Sigmoid)
                    covs.append(cv)

                vregs = [nc.vector.alloc_register(f"offv_{g8}_{i}")
                         for i in range(8)]
                liv = nc.vector.reg_load(vregs, offs_sb[0:1, g8:g8 + 8])
                if prev_v is not None:
                    add_dep_helper(liv.ins, prev_v.ins, sync=False,
                                   reason="reg pressure")
                voff = [nc.vector.snap(rg, donate=True,
                                       min_val=0, max_val=W - WIN)
                        for rg in vregs]
                gregs = [nc.gpsimd.alloc_register(f"offg_{g8}_{i}")
                         for i in range(8)]
                lig = nc.gpsimd.reg_load(gregs, offs_sb[0:1, g8:g8 + 8])
                if prev_g is not None:
                    add_dep_helper(lig.ins, prev_g.ins, sync=False,
                                   reason="reg pressure")
                goff = [nc.gpsimd.snap(rg, donate=True,
                                       min_val=0, max_val=W - WIN)
                        for rg in gregs]

                for i in range(8):
                    k = g8 + i
                    cov = covs[i // 2][:, (i % 2) * WIN:(i % 2) * WIN + WIN]
                    # DVE order: w_k, R_{k-1}, T_k, G_{k-1}
                    tw = T[:, bass.ds(voff[i], WIN)]
                    w = wpool.tile([ROWS, WIN], F16)
                    nc.vector.tensor_tensor(w, tw, cov, OP.mult)
                    tmpb = bpool.tile([ROWS, WIN], F16)
                    nc.scalar.activation(
                        tmpb, w, AF.Copy,
                        scale=scal_sb[:, 4 * k + 3:4 * k + 4])
                    cbw = CB[:, bass.ds(goff[i], WIN)]
                    prev_g = nc.gpsimd.tensor_tensor(cbw, cbw, tmpb, OP.add)
                    wp, kp, offp = (pend if pend is not None
                                    else (None, None, None))
                    if wp is not None:
                        crw = CR[:, bass.ds(offp, WIN)]
                        nc.vector.scalar_tensor_tensor(
                            crw, wp, scal_sb[:, 4 * kp + 1:4 * kp + 2], crw,
                            OP.mult, OP.add)
                    nc.vector.scalar_tensor_tensor(
                        tw, w, scal_sb[:, 4 * k + 0:4 * k + 1], tw,
                        OP.mult, OP.add)
                    if wp is not None:
                        cgw = CG[:, bass.ds(offp, WIN)]
                        prev_v = nc.vector.scalar_tensor_tensor(
                            cgw, wp, scal_sb[:, 4 * kp + 2:4 * kp + 3], cgw,
                            OP.mult, OP.add)
                    pend = (w, k, voff[i])

            # flush the last circle's R/G MACs
            if pend is not None:
                wp, kp, offp = pend
                crw = CR[:, bass.ds(offp, WIN)]
                nc.vector.scalar_tensor_tensor(
                    crw, wp, scal_sb[:, 4 * kp + 1:4 * kp + 2], crw,
                    OP.mult, OP.add)
                cgw = CG[:, bass.ds(offp, WIN)]
                nc.vector.scalar_tensor_tensor(
                    cgw, wp, scal_sb[:, 4 * kp + 2:4 * kp + 3], cgw,
                    OP.mult, OP.add)
                pend = None

        # ---------- store raw planes; host interleaves + computes alpha ---
        nc.sync.dma_start(out_d[:, 0 * W:1 * W], CR)
        nc.sync.dma_start(out_d[:, 1 * W:2 * W], CG)
        nc.sync.dma_start(out_d[:, 2 * W:3 * W], CB)
        nc.sync.dma_start(out_d[:, 3 * W:4 * W], T)

    nc.compile()
    return nc


_CACHE = {}


def _get_nc(nc_slots):
    if nc_slots not in _CACHE:
        _CACHE[nc_slots] = build_nc(nc_slots)
    return _CACHE[nc_slots]


def kernel(centers, radii, colors):
    centers = np.asarray(centers, np.float32)
    radii = np.asarray(radii, np.float32)
    colors = np.asarray(colors, np.float32)

    counts = []
    for core in range(N_CORES):
        idx, *_ = _build_core_inputs(centers, radii, colors, core)
        counts.append(len(idx))
    nc_slots = max(8, ((max(counts) + 7) // 8) * 8)

    nc = _get_nc(nc_slots)
    ins = make_inputs(centers, radii, colors, nc_slots)
    res = bass_utils.run_bass_kernel_spmd(nc, ins, list(range(N_CORES)),
                                          trace=False)
    out = np.zeros((H, W, 4), np.float32)
    for c in range(N_CORES):
        planes = np.asarray(res.results[c]["out"]).astype(np.float32)
        planes = planes.reshape(ROWS, 4, W)
        sl = slice(c * ROWS, (c + 1) * ROWS)
        out[sl, :, 0] = planes[:, 0]
        out[sl, :, 1] = planes[:, 1]
        out[sl, :, 2] = planes[:, 2]
        out[sl, :, 3] = 1.0 - planes[:, 3]
    return out


# revision 11
# speedup vs baseline: 1.0285x; 1.0079x over previous
"""DiffVG-style circle renderer on 8 Trainium2 NeuronCores.

Strategy: shard the 1024x1024 image by rows (128 rows per core). Each core
composites only the circles whose vertical span intersects its row band,
processing each circle front-to-back on a 224-column window around its
center:

    cov = sigmoid(arg)            arg ~= 2*(r - sqrt(dy^2+dx^2))  (see below)
    w   = T * cov                 (T = transmittance plane, init 1)
    C  += w * (alpha * color);  T -= alpha * w
Final:  rgb = C, a = 1 - T  (identical to the sequential 'over' scan).

The sigmoid argument is approximated per circle by an affine function of
t = d^2:  arg = a*t + b  with (a, b) least-max fitted on host against
2*(r - sqrt(t)).  This lets the PE matmul produce the *finished* sigmoid
argument (scale and bias folded into the operands), so ACT needs exactly
one table (sigmoid), no sqrt pass, and reads PSUM directly.

Engine split per circle:
  PE     arg = Ay[p] (+) Bx[x] outer-sum; two circles per matmul via a K=8
         block-diagonal layout (recentred hi/lo bf16 splits keep the
         argument accurate to ~6e-3)
  ACT    sigmoid straight from PSUM, one pair (448 cols) per op, bf16 out
  DVE    w = T*cov, T-MAC, R-MAC, G-MAC on dynamic 224-px windows (fp16
         2x mode; even window offsets keep 4B alignment), ordered
         [w_k, R_{k-1}, T_k, G_{k-1}] to pad same-engine RAW interlocks
  GPSIMD B-MAC (scalar_tensor_tensor)
Composite scalars come from a small per-core scal tensor (per-partition
scalar APs); the finish pass (RGBA interleave + alpha) runs on the host
from the 4 raw fp16 planes.
"""

import sys

if "/opt/trn_rl_repo" not in sys.path:
    sys.path.insert(0, "/opt/trn_rl_repo")

import numpy as np
import ml_dtypes

import concourse.bass as bass
import concourse.bacc as bacc
import concourse.mybir as mybir
from concourse.tile import TileContext, add_dep_helper
from concourse import bass_utils

H = 1024
W = 1024
ROWS = 128          # rows per core
N_CORES = 8
WIN = 224           # column window per circle (covers 2*(r+9) for r < 100)
MARGIN = 9.0
F32 = mybir.dt.float32
BF16 = mybir.dt.bfloat16
F16 = mybir.dt.float16
AF = mybir.ActivationFunctionType
OP = mybir.AluOpType
BF = ml_dtypes.bfloat16


def _build_core_inputs(centers, radii, colors, core):
    """Per-core circle list (slots ordered top-circle-first)."""
    y0 = ROWS * core
    cy = centers[:, 1].astype(np.float64)
    cx = centers[:, 0].astype(np.float64)
    r = radii.astype(np.float64)
    keep = (cy + r + MARGIN >= y0 + 0.5) & (cy - r - MARGIN <= y0 + ROWS - 0.5)
    idx = np.where(keep)[0][::-1]  # reversed: topmost (last-drawn) first
    return idx, cx[idx], cy[idx], r[idx], colors[idx].astype(np.float64)


def _hilo(x):
    hi = x.astype(BF)
    lo = (x - hi.astype(np.float64)).astype(BF)
    return hi, lo


def _fit_affine(r):
    """Least-max fit of sigmoid(a*t+b) to sigmoid(2*(r-sqrt(t))), t=d^2."""
    d = np.linspace(-11.0, 11.0, 353)
    t = (r + d) ** 2
    tgt = 1.0 / (1.0 + np.exp(2.0 * d))
    a0, b0 = -1.0 / r, r

    def err(u, v):
        arg = np.multiply.outer(u, a0 * t) + (b0 * u + v)[:, None]
        cov = 1.0 / (1.0 + np.exp(-arg))
        return np.abs(cov - tgt[None, :]).max(axis=1)

    u, v = 1.0, 0.0
    for span, n in ((0.06, 13), (0.012, 9), (0.002, 9)):
        us = u + np.linspace(-span, span, n)
        vs = v + np.linspace(-span * 8 * r / 20, span * 8 * r / 20, n)
        uu, vv = np.meshgrid(us, vs, indexing="ij")
        e = err(uu.ravel(), vv.ravel())
        k = int(np.argmin(e))
        u, v = uu.ravel()[k], vv.ravel()[k]
    return a0 * u, b0 * u + v


def make_inputs(centers, radii, colors, nc_slots):
    assert nc_slots % 8 == 0
    ins = []
    for core in range(N_CORES):
        y0 = ROWS * core
        idx, cx, cy, r, col = _build_core_inputs(centers, radii, colors, core)
        n = len(idx)
        assert n <= nc_slots
        offs = np.zeros((1, nc_slots), np.int32)
        # per-circle scalars, replicated down partitions:
        # [4k+0]=-alpha, [4k+1]=alpha*cr, [4k+2]=alpha*cg, [4k+3]=alpha*cb
        scal = np.zeros((ROWS, nc_slots * 4), np.float32)
        # two circles (a, b) share one K=8 matmul: lhsT rows 0-3 belong to
        # a, rows 4-7 to b; rhs zero-masks the other circle's columns.
        lhsT = np.zeros((8, (nc_slots // 2) * ROWS), BF)
        rhs = np.zeros((8, nc_slots * WIN), BF)

        p = np.arange(ROWS, dtype=np.float64)
        j = np.arange(WIN, dtype=np.float64)
        for k in range(n):
            off = int(np.clip(np.floor(cx[k]) - 112.0, 0.0, float(W - WIN)))
            off &= ~1  # even: keeps bf16 windows 4B-aligned for 2x DVE mode
            offs[0, k] = off
            a_k, b_k = _fit_affine(r[k])
            ya = a_k * (y0 + p + 0.5 - cy[k]) ** 2 + b_k
            xa = a_k * (off + j + 0.5 - cx[k]) ** 2
            # recentre so each hi/lo bf16 split carries a small range
            c = -0.5 * (ya.max() + ya.min())
            ya = ya + c
            xa = xa - c
            alpha = col[k, 3]
            scal[:, 4 * k + 0] = -alpha
            scal[:, 4 * k + 1] = alpha * col[k, 0]
            scal[:, 4 * k + 2] = alpha * col[k, 1]
            scal[:, 4 * k + 3] = alpha * col[k, 2]
            yh, yl = _hilo(ya)
            xh, xl = _hilo(xa)
            pair, half = divmod(k, 2)
            rbase = 4 * half
            ls = slice(pair * ROWS, (pair + 1) * ROWS)
            lhsT[rbase + 0, ls] = yh
            lhsT[rbase + 1, ls] = yl
            lhsT[rbase + 2, ls] = 1.0
            lhsT[rbase + 3, ls] = 1.0
            rs = slice(k * WIN, (k + 1) * WIN)
            rhs[rbase + 0, rs] = 1.0
            rhs[rbase + 1, rs] = 1.0
            rhs[rbase + 2, rs] = xh
            rhs[rbase + 3, rs] = xl
        ins.append({"offs": offs, "scal": scal, "lhsT": lhsT, "rhs": rhs})
    return ins


def build_nc(nc_slots):
    assert nc_slots % 8 == 0
    nc = bacc.Bacc("TRN2", target_bir_lowering=False, debug=False,
                   num_devices=N_CORES)
    offs_d = nc.dram_tensor("offs", [1, nc_slots], mybir.dt.int32,
                            kind="ExternalInput").ap()
    scal_d = nc.dram_tensor("scal", [ROWS, nc_slots * 4], F32,
                            kind="ExternalInput").ap()
    lhsT_d = nc.dram_tensor("lhsT", [8, (nc_slots // 2) * ROWS], BF16,
                            kind="ExternalInput").ap()
    rhs_d = nc.dram_tensor("rhs", [8, nc_slots * WIN], BF16,
                           kind="ExternalInput").ap()
    out_d = nc.dram_tensor("out", [ROWS, W * 4], F16,
                           kind="ExternalOutput").ap()

    with TileContext(nc) as tc:
        # persistent state (bf16 planes)
        T = nc.alloc_sbuf_tensor("T", [ROWS, W], F16).ap()
        CR = nc.alloc_sbuf_tensor("CR", [ROWS, W], F16).ap()
        CG = nc.alloc_sbuf_tensor("CG", [ROWS, W], F16).ap()
        CB = nc.alloc_sbuf_tensor("CB", [ROWS, W], F16).ap()
        offs_sb = nc.alloc_sbuf_tensor("offs_sb", [1, nc_slots],
                                       mybir.dt.int32).ap()
        scal_sb = nc.alloc_sbuf_tensor("scal_sb", [ROWS, nc_slots * 4],
                                       F32).ap()

        nc.sync.dma_start(offs_sb, offs_d)
        nc.sync.dma_start(scal_sb, scal_d)
        nc.vector.memset(T, 1.0)
        nc.vector.memset(CR, 0.0)
        nc.gpsimd.memset(CG, 0.0)
        nc.gpsimd.memset(CB, 0.0)

        with (
            tc.tile_pool(name="psum", bufs=8, space="PSUM") as psum_pool,
            tc.tile_pool(name="ops", bufs=4) as oppool,
            tc.tile_pool(name="cov", bufs=9) as covpool,
            tc.tile_pool(name="w", bufs=6) as wpool,
            tc.tile_pool(name="tmpb", bufs=6) as bpool,
        ):
            prev_v = None
            prev_g = None
            # pending R/G MAC for the previous circle (emitted one circle
            # late so the serial T chain never reads a value written by
            # the immediately preceding DVE instruction)
            pend = None

            def front(g8):
                """DMA + matmuls + per-pair sigmoids for one group of 8."""
                lh_t = oppool.tile([8, 4 * ROWS], BF16, tag="lh")
                rh_t = oppool.tile([8, 8 * WIN], BF16, tag="rh")
                p0 = g8 // 2
                nc.sync.dma_start(lh_t, lhsT_d[:, p0 * ROWS:(p0 + 4) * ROWS])
                nc.sync.dma_start(rh_t, rhs_d[:, g8 * WIN:(g8 + 8) * WIN])
                covs = []
                for i in range(4):
                    pt = psum_pool.tile([ROWS, 2 * WIN], F32)
                    nc.tensor.matmul(
                        pt,
                        lh_t[:, i * ROWS:(i + 1) * ROWS],
                        rh_t[:, i * 2 * WIN:(i + 1) * 2 * WIN],
                        start=True, stop=True)
                    cv = covpool.tile([ROWS, 2 * WIN], F16)
                    nc.scalar.activation(cv, pt, AF.Sigmoid)
                    covs.append(cv)
                return covs

            covs_next = front(0)
            for g8 in range(0, nc_slots, 8):
                covs = covs_next
                if g8 + 8 < nc_slots:
                    covs_next = front(g8 + 8)

                vregs = [nc.vector.alloc_register(f"offv_{g8}_{i}")
                         for i in range(8)]
                liv = nc.vector.reg_load(vregs, offs_sb[0:1, g8:g8 + 8])
                if prev_v is not None:
                    add_dep_helper(liv.ins, prev_v.ins, sync=False,
                                   reason="reg pressure")
                voff = [nc.vector.snap(rg, donate=True,
                                       min_val=0, max_val=W - WIN)
                        for rg in vregs]
                gregs = [nc.gpsimd.alloc_register(f"offg_{g8}_{i}")
                         for i in range(8)]
                lig = nc.gpsimd.reg_load(gregs, offs_sb[0:1, g8:g8 + 8])
                if prev_g is not None:
                    add_dep_helper(lig.ins, prev_g.ins, sync=False,
                                   reason="reg pressure")
                goff = [nc.gpsimd.snap(rg, donate=True,
                                       min_val=0, max_val=W - WIN)
                        for rg in gregs]

                for i in range(8):
                    k = g8 + i
                    cov = covs[i // 2][:, (i % 2) * WIN:(i % 2) * WIN + WIN]
                    # DVE order: w_k, R_{k-1}, T_k, G_{k-1}
                    tw = T[:, bass.ds(voff[i], WIN)]
                    w = wpool.tile([ROWS, WIN], F16)
                    nc.vector.tensor_tensor(w, tw, cov, OP.mult)
                    tmpb = bpool.tile([ROWS, WIN], F16)
                    nc.scalar.activation(
                        tmpb, w, AF.Copy,
                        scale=scal_sb[:, 4 * k + 3:4 * k + 4])
                    cbw = CB[:, bass.ds(goff[i], WIN)]
                    prev_g = nc.gpsimd.tensor_tensor(cbw, cbw, tmpb, OP.add)
                    wp, kp, offp = (pend if pend is not None
                                    else (None, None, None))
                    if wp is not None:
                        crw = CR[:, bass.ds(offp, WIN)]
                        nc.vector.scalar_tensor_tensor(
                            crw, wp, scal_sb[:, 4 * kp + 1:4 * kp + 2], crw,
                            OP.mult, OP.add)
                    nc.vector.scalar_tensor_tensor(
                        tw, w, scal_sb[:, 4 * k + 0:4 * k + 1], tw,
                        OP.mult, OP.add)
                    if wp is not None:
                        cgw = CG[:, bass.ds(offp, WIN)]
                        prev_v = nc.vector.scalar_tensor_tensor(
                            cgw, wp, scal_sb[:, 4 * kp + 2:4 * kp + 3], cgw,
                            OP.mult, OP.add)
                    pend = (w, k, voff[i])

            # flush the last circle's R/G MACs
            if pend is not None:
                wp, kp, offp = pend
                crw = CR[:, bass.ds(offp, WIN)]
                nc.vector.scalar_tensor_tensor(
                    crw, wp, scal_sb[:, 4 * kp + 1:4 * kp + 2], crw,
                    OP.mult, OP.add)
                cgw = CG[:, bass.ds(offp, WIN)]
                nc.vector.scalar_tensor_tensor(
                    cgw, wp, scal_sb[:, 4 * kp + 2:4 * kp + 3], cgw,
                    OP.mult, OP.add)
                pend = None

        # ---------- store raw planes; host interleaves + computes alpha ---
        nc.sync.dma_start(out_d[:, 0 * W:1 * W], CR)
        nc.sync.dma_start(out_d[:, 1 * W:2 * W], CG)
        nc.sync.dma_start(out_d[:, 2 * W:3 * W], CB)
        nc.sync.dma_start(out_d[:, 3 * W:4 * W], T)

    nc.compile()
    return nc


_CACHE = {}


def _get_nc(nc_slots):
    if nc_slots not in _CACHE:
        _CACHE[nc_slots] = build_nc(nc_slots)
    return _CACHE[nc_slots]


def kernel(centers, radii, colors):
    centers = np.asarray(centers, np.float32)
    radii = np.asarray(radii, np.float32)
    colors = np.asarray(colors, np.float32)

    counts = []
    for core in range(N_CORES):
        idx, *_ = _build_core_inputs(centers, radii, colors, core)
        counts.append(len(idx))
    nc_slots = max(8, ((max(counts) + 7) // 8) * 8)

    nc = _get_nc(nc_slots)
    ins = make_inputs(centers, radii, colors, nc_slots)
    res = bass_utils.run_bass_kernel_spmd(nc, ins, list(range(N_CORES)),
                                          trace=False)
    out = np.zeros((H, W, 4), np.float32)
    for c in range(N_CORES):
        planes = np.asarray(res.results[c]["out"]).astype(np.float32)
        planes = planes.reshape(ROWS, 4, W)
        sl = slice(c * ROWS, (c + 1) * ROWS)
        out[sl, :, 0] = planes[:, 0]
        out[sl, :, 1] = planes[:, 1]
        out[sl, :, 2] = planes[:, 2]
        out[sl, :, 3] = 1.0 - planes[:, 3]
    return out
